# revision 1
# baseline (speedup 1.0000x reference)
"""Trainium2 Bass kernel for nn_CPWGenerator (B=16384, D=128, P=10, F=1024).

Data-parallel over batch across 8 NeuronCores (2048 rows/core). Per core:
  - feature-major 3-layer MLPs (control-point head + weight head)
  - softmax denominator cancels: out = (basis @ (e*cpm)) / (basis @ e)
    with e = exp(logits) raw (scale-invariant; the reference's +1e-8 eps
    term shifts the result by <1.1e-7 of scale here — measured — because
    den >= 0.07)
  - final basis matmuls produce batch-major [128, F] output tiles directly
  - division: reciprocal_approx_fast (DVE) + multiply (DVE/GPSIMD split)
Matmuls run as float32r (fp32 storage, 11-bit-mantissa operand rounding,
exact fp32 accumulation) at full PE rate.
"""
import sys
if "/opt/trn_rl_repo" not in sys.path:
    sys.path.insert(0, "/opt/trn_rl_repo")

from contextlib import ExitStack

import numpy as np

import concourse.bacc as bacc
import concourse.mybir as mybir
import concourse.tile as tile
from concourse.bass_utils import run_bass_kernel_spmd

F32 = mybir.dt.float32
F32R = mybir.dt.float32r
AF = mybir.ActivationFunctionType

# problem shapes (hardcoded per contest contract)
B, D, P, F = 16384, 128, 10, 1024
NCORES = 8
BC = B // NCORES          # rows per core = 2048
BLOCKS = [512, 512, 512, 512]   # batch blocks (sum = BC)
EPS = 1e-8

# (block, j) pairs whose final multiply runs on GPSIMD (ACT copies the
# numerator out of PSUM first); the rest multiply on DVE straight from PSUM.
GP_MUL = {(0, 1), (0, 3), (1, 1), (1, 3),
          (2, 1), (2, 3), (3, 1), (3, 2), (3, 3)}
# (block, j) pairs whose output DMA issues from GPSIMD (SWDGE) instead of
# the SP HWDGE queue, to spread DMA issue across queues.
GP_DMA = set()

# f32r const blob column offsets
_C_W1T = 0            # [128 x 128]
_C_W2T = 128          # [128 x 256]
_C_W3T = 384          # [128 x 40]  (W3Ta | W3Tb, 20 cols each)
_C_WW1T = 424         # [128 x 64]
_C_WW2T = 488         # [64  x 128]
_C_WW3T = 616         # [128 x 10]
_C_P20 = 626          # [20  x 10]  pairing matrix (0.5 per pair)
C_R = 636

# fp32 const blob columns
_C_ID = 0             # [128 x 128] identity
_C_B1 = 128
_C_B2A = 129
_C_B2B = 130
_C_B3 = 131
_C_WB1 = 132
_C_WB2 = 133
_C_WB3 = 134
C_F = 135


def round_f32r(x: np.ndarray) -> np.ndarray:
    """fp32 -> fp32r rounding (keep 11 explicit mantissa bits, RNE).
    Matches TRN2 hardware exactly (validated on device)."""
    u = np.ascontiguousarray(x, dtype=np.float32).view(np.uint32)
    keep = np.uint32(0xFFFFF000)
    half = np.uint32(0x800)
    lsb = (u >> np.uint32(12)) & np.uint32(1)
    r = (u + half - np.uint32(1) + lsb) & keep
    return r.view(np.float32)


def basis_matrix() -> np.ndarray:
    """Replicates reference._basis_matrix in float32."""
    t = np.linspace(0.0, 1.0, F, dtype=np.float32)
    centers = (np.arange(P, dtype=np.float32) / np.float32(P - 1))
    sigma = np.float32(1.0 / P)
    z = (t[:, None] - centers[None, :]).astype(np.float32)
    basis = np.exp(-(z * z) / (np.float32(2.0) * sigma * sigma),
                   dtype=np.float32)
    return basis / (basis.sum(axis=1, keepdims=True) + np.float32(EPS))


def build_program():
    nc = bacc.Bacc()
    x_in = nc.declare_dram_parameter("x", [BC, D], F32, isOutput=False)
    wr_in = nc.declare_dram_parameter("wr", [128, C_R], F32R, isOutput=False)
    bt_in = nc.declare_dram_parameter("bt", [P, F], F32R, isOutput=False)
    wf_in = nc.declare_dram_parameter("wf", [128, C_F], F32, isOutput=False)
    out = nc.declare_dram_parameter("out", [BC, F], F32, isOutput=True)

    with tile.TileContext(nc) as tc, ExitStack() as ctx:
        cpool = ctx.enter_context(tc.tile_pool(name="const", bufs=1))
        wpool = ctx.enter_context(tc.tile_pool(name="work", bufs=2))
        npool = ctx.enter_context(tc.tile_pool(name="numcp", bufs=2))
        rpool = ctx.enter_context(tc.tile_pool(name="recip", bufs=2))
        opool = ctx.enter_context(tc.tile_pool(name="outp", bufs=4))
        ppool = ctx.enter_context(tc.tile_pool(name="psum", bufs=4, space="PSUM"))
        qpool = ctx.enter_context(tc.tile_pool(name="psumo", bufs=2, space="PSUM"))

        wr = cpool.tile([128, C_R], F32R)
        bt = cpool.tile([P, F], F32R)
        wf = cpool.tile([128, C_F], F32)
        xall = cpool.tile([128, BC], F32)

        def x_dma(xoff, nb_):
            nc.gpsimd.dma_start(
                xall[:, xoff:xoff + nb_].rearrange(
                    "p (c d) -> p c d", c=nb_ // 128),
                x_in[xoff:xoff + nb_, :].rearrange(
                    "(c p) d -> p c d", p=128),
            )

        # in-DMA order tuned for pipeline fill: identity+biases first (gates
        # the first transpose), then x block 0, then weights, then the rest
        nc.gpsimd.dma_start(wf[:], wf_in[:])
        x_dma(0, BLOCKS[0])
        nc.gpsimd.dma_start(wr[:], wr_in[:])
        nc.gpsimd.dma_start(bt[:], bt_in[:])
        xoff = BLOCKS[0]
        for nb_ in BLOCKS[1:]:
            x_dma(xoff, nb_)
            xoff += nb_

        ident = wf[:, _C_ID:_C_ID + 128]

        def mm(out_ap, lhsT, rhs, start=True, stop=True):
            nc.tensor.matmul(out_ap, lhsT, rhs, start=start, stop=stop)

        x0 = 0
        for blk, NB in enumerate(BLOCKS):

            # --- transpose x block: [128b,128d] chunks -> xT [128d, NB b]
            xtp = ppool.tile([128, NB], F32, tag="ps")
            for c in range(NB // 128):
                nc.tensor.matmul(
                    xtp[:, 128 * c:128 * (c + 1)],
                    xall[:, x0 + 128 * c:x0 + 128 * (c + 1)],
                    ident,
                    is_transpose=True,
                    start=(c % 4 == 0),
                    stop=(c % 4 == 3),
                )
            xt = wpool.tile([128, NB], F32R)
            nc.scalar.activation(xt[:], xtp[:], AF.Copy)

            # --- cp MLP (feature-major)
            h1p = ppool.tile([128, NB], F32, tag="ps")
            for n in range(NB // 512):
                mm(h1p[:, 512 * n:512 * (n + 1)],
                   wr[:, _C_W1T:_C_W1T + 128],
                   xt[:, 512 * n:512 * (n + 1)])
            h1 = wpool.tile([128, NB], F32R)
            nc.scalar.activation(h1[:], h1p[:], AF.Relu,
                                 bias=wf[:, _C_B1:_C_B1 + 1])

            h2pa = ppool.tile([128, NB], F32, tag="ps")
            for n in range(NB // 512):
                mm(h2pa[:, 512 * n:512 * (n + 1)],
                   wr[:, _C_W2T:_C_W2T + 128],
                   h1[:, 512 * n:512 * (n + 1)])
            h2a = wpool.tile([128, NB], F32R)
            nc.scalar.activation(h2a[:], h2pa[:], AF.Relu,
                                 bias=wf[:, _C_B2A:_C_B2A + 1])

            h2pb = ppool.tile([128, NB], F32, tag="ps")
            for n in range(NB // 512):
                mm(h2pb[:, 512 * n:512 * (n + 1)],
                   wr[:, _C_W2T + 128:_C_W2T + 256],
                   h1[:, 512 * n:512 * (n + 1)])
            h2b = wpool.tile([128, NB], F32R)
            nc.scalar.activation(h2b[:], h2pb[:], AF.Relu,
                                 bias=wf[:, _C_B2B:_C_B2B + 1])

            cpp = ppool.tile([20, NB], F32, tag="ps")
            for n in range(NB // 512):
                sl = slice(512 * n, 512 * (n + 1))
                mm(cpp[:, sl], wr[:, _C_W3T:_C_W3T + 20], h2a[:, sl],
                   stop=False)
                mm(cpp[:, sl], wr[:, _C_W3T + 20:_C_W3T + 40], h2b[:, sl],
                   start=False, stop=True)
            cp = wpool.tile([20, NB], F32R)
            nc.scalar.activation(cp[:], cpp[:], AF.Tanh,
                                 bias=wf[0:20, _C_B3:_C_B3 + 1])

            # --- w MLP
            g1p = ppool.tile([64, NB], F32, tag="ps")
            for n in range(NB // 512):
                mm(g1p[:, 512 * n:512 * (n + 1)],
                   wr[:, _C_WW1T:_C_WW1T + 64],
                   xt[:, 512 * n:512 * (n + 1)])
            g1 = wpool.tile([64, NB], F32R)
            nc.scalar.activation(g1[:], g1p[:], AF.Relu,
                                 bias=wf[0:64, _C_WB1:_C_WB1 + 1])

            g2p = ppool.tile([128, NB], F32, tag="ps")
            for n in range(NB // 512):
                mm(g2p[:, 512 * n:512 * (n + 1)],
                   wr[0:64, _C_WW2T:_C_WW2T + 128],
                   g1[:, 512 * n:512 * (n + 1)])
            g2 = wpool.tile([128, NB], F32R)
            nc.scalar.activation(g2[:], g2p[:], AF.Relu,
                                 bias=wf[:, _C_WB2:_C_WB2 + 1])

            wlp = ppool.tile([10, NB], F32, tag="ps")
            for n in range(NB // 512):
                mm(wlp[:, 512 * n:512 * (n + 1)],
                   wr[:, _C_WW3T:_C_WW3T + 10],
                   g2[:, 512 * n:512 * (n + 1)])
            e = wpool.tile([10, NB], F32R)
            nc.scalar.activation(e[:], wlp[:], AF.Exp,
                                 bias=wf[0:10, _C_WB3:_C_WB3 + 1])

            # --- pairing: cp_mean = P20.T @ cp -> [10, NB]
            pairp = ppool.tile([10, NB], F32, tag="ps")
            for n in range(NB // 512):
                sl = slice(512 * n, 512 * (n + 1))
                mm(pairp[:, sl], wr[0:20, _C_P20:_C_P20 + 10], cp[:, sl])

            # num lhsT rows: e * cp_mean  (DVE, psum x sbuf)
            wcpmN = wpool.tile([10, NB], F32R)
            nc.vector.tensor_mul(wcpmN[:], pairp[:], e[:].bitcast(F32))

            # --- output M-blocks (den emitted first so recip(j+1) can
            # overlap mul(j) with only 2 psum slots)
            for j in range(NB // 128):
                bsl = slice(128 * j, 128 * (j + 1))
                denp = qpool.tile([128, F], F32, tag="out")
                for h in range(F // 512):
                    fsl = slice(512 * h, 512 * (h + 1))
                    mm(denp[:, fsl], e[:, bsl], bt[:, fsl])
                nump = qpool.tile([128, F], F32, tag="out")
                for h in range(F // 512):
                    fsl = slice(512 * h, 512 * (h + 1))
                    mm(nump[:, fsl], wcpmN[:, bsl], bt[:, fsl])
                r = rpool.tile([128, F], F32)
                nc.vector.reciprocal_approx_fast(out=r[:], in_=denp[:])
                o = opool.tile([128, F], F32)
                if (blk, j) in GP_MUL:
                    numS = npool.tile([128, F], F32)
                    nc.scalar.copy(numS[:], nump[:])
                    nc.gpsimd.tensor_mul(o[:], numS[:], r[:])
                else:
                    nc.vector.tensor_mul(o[:], nump[:], r[:])
                dma_eng = nc.gpsimd if (blk, j) in GP_DMA else nc.sync
                dma_eng.dma_start(out[x0 + 128 * j:x0 + 128 * (j + 1), :],
                                  o[:])
            x0 += NB

    nc.compile()
    return nc


def host_consts(cp_w1, cp_b1, cp_w2, cp_b2, cp_w3, cp_b3,
                w_w1, w_b1, w_w2, w_b2, w_w3, w_b3):
    basis = basis_matrix()                     # [F, P]

    wr = np.zeros((128, C_R), np.float32)
    wr[:, _C_W1T:_C_W1T + 128] = cp_w1.T       # [128,128]
    wr[:, _C_W2T:_C_W2T + 256] = cp_w2.T       # [128,256]
    w3t = cp_w3.T                              # [256,20]
    wr[:, _C_W3T:_C_W3T + 20] = w3t[0:128]
    wr[:, _C_W3T + 20:_C_W3T + 40] = w3t[128:256]
    wr[:, _C_WW1T:_C_WW1T + 64] = w_w1.T       # [128,64]
    wr[0:64, _C_WW2T:_C_WW2T + 128] = w_w2.T   # [64,128]
    wr[:, _C_WW3T:_C_WW3T + 10] = w_w3.T       # [128,10]
    p20 = np.zeros((20, 10), np.float32)
    for p in range(P):
        p20[2 * p, p] = 0.5
        p20[2 * p + 1, p] = 0.5
    wr[0:20, _C_P20:_C_P20 + 10] = p20
    wr = round_f32r(wr)

    bt = round_f32r(np.ascontiguousarray(basis.T))   # [P, F]

    wf = np.zeros((128, C_F), np.float32)
    wf[:, _C_ID:_C_ID + 128] = np.eye(128, dtype=np.float32)
    wf[:, _C_B1] = cp_b1
    wf[:, _C_B2A] = cp_b2[0:128]
    wf[:, _C_B2B] = cp_b2[128:256]
    wf[0:20, _C_B3] = cp_b3
    wf[0:64, _C_WB1] = w_b1
    wf[:, _C_WB2] = w_b2
    wf[0:10, _C_WB3] = w_b3
    return wr, bt, wf


_NC_CACHE = None


def get_program():
    global _NC_CACHE
    if _NC_CACHE is None:
        _NC_CACHE = build_program()
    return _NC_CACHE


def kernel(x, cp_w1, cp_b1, cp_w2, cp_b2, cp_w3, cp_b3,
           w_w1, w_b1, w_w2, w_b2, w_w3, w_b3, _return_raw=False):
    x = np.asarray(x, np.float32)
    wr, bt, wf = host_consts(
        np.asarray(cp_w1, np.float32), np.asarray(cp_b1, np.float32),
        np.asarray(cp_w2, np.float32), np.asarray(cp_b2, np.float32),
        np.asarray(cp_w3, np.float32), np.asarray(cp_b3, np.float32),
        np.asarray(w_w1, np.float32), np.asarray(w_b1, np.float32),
        np.asarray(w_w2, np.float32), np.asarray(w_b2, np.float32),
        np.asarray(w_w3, np.float32), np.asarray(w_b3, np.float32))

    nc = get_program()
    in_maps = [
        {"x": np.ascontiguousarray(x[i * BC:(i + 1) * BC]),
         "wr": wr, "bt": bt, "wf": wf}
        for i in range(NCORES)
    ]
    res = run_bass_kernel_spmd(nc, in_maps, list(range(NCORES)))
    outs = [res.results[i]["out"] for i in range(NCORES)]
    full = np.concatenate(outs, axis=0)
    if _return_raw:
        return full, res
    return full



# revision 8
# speedup vs baseline: 1.1299x; 1.1299x over previous
"""Trainium2 Bass kernel for nn_CPWGenerator (B=16384, D=128, P=10, F=1024).

Data-parallel over batch across 8 NeuronCores (2048 rows/core). Per core:
  - feature-major 3-layer MLPs (control-point head + weight head)
  - softmax denominator cancels: out = (basis @ (e*cpm)) / (basis @ e)
  - KEY RESTRUCTURE vs the first version: the ratio num/den is evaluated at
    only S=64 coarse t-samples (basis row-normalization cancels in the
    ratio), then upsampled to F=1024 by a single PE matmul against a
    precomputed cubic-Lagrange interpolation matrix im [S, F]. The ratio is
    a sum of 10 Gaussians with sigma ~ 102 grid points, so cubic
    interpolation from 64 uniform samples adds < 2e-4 relative error
    (measured end-to-end: 5.2e-4 total vs reference, gate is 2e-2).
    This cuts the full-resolution elementwise work (reciprocal + multiply
    at [128, 1024] per tile) down to one divide at [64, 512] per block.
  - PSUM->SBUF evictions are spread across ACT, DVE and GPSIMD; output DMA
    is spread across the SP, ACT and GPSIMD queues.
Matmuls run as float32r (fp32 storage, 11-bit-mantissa operand rounding,
exact fp32 accumulation) at full PE rate.
"""
import sys
if "/opt/trn_rl_repo" not in sys.path:
    sys.path.insert(0, "/opt/trn_rl_repo")

from contextlib import ExitStack

import numpy as np

import concourse.bacc as bacc
import concourse.mybir as mybir
import concourse.tile as tile
from concourse.bass_utils import run_bass_kernel_spmd

F32 = mybir.dt.float32
F32R = mybir.dt.float32r
AF = mybir.ActivationFunctionType
ALU = mybir.AluOpType

# problem shapes (hardcoded per contest contract)
B, D, P, F = 16384, 128, 10, 1024
NCORES = 8
BC = B // NCORES          # rows per core = 2048
NB = 512                  # batch block
NBLK = BC // NB           # 4
S = 64                    # coarse t-samples for the ratio
EPS = 1e-8

# f32r const blob (wr) column offsets
_C_W1T = 0            # [128 x 128]
_C_W2T = 128          # [128 x 256]
_C_W3T = 384          # [128 x 40]  (W3Ta | W3Tb, 20 cols each)
_C_WW1T = 424         # [128 x 64]
_C_WW2T = 488         # [64  x 128]
_C_WW3T = 616         # [128 x 10]
_C_P20 = 626          # [20  x 10]  pairing matrix (0.5 per pair)
_C_BTC = 636          # [10  x 64]  unnormalized basis at the S t-samples
_C_IDR = 700          # [128 x 128] identity (for PE transpose)
C_R = 828

# fp32 bias blob (wf) columns
_C_B1 = 0
_C_B2A = 1
_C_B2B = 2
_C_B3 = 3             # rows 0-19
_C_WB1 = 4            # rows 0-63
_C_WB2 = 5
_C_WB3 = 6            # rows 0-9
C_F = 7

# ---- engine assignment tables (tunable) ----
# (GPSIMD cannot access PSUM — BIR verifier — so evictions are ACT/DVE only.)
# MLP eviction engine per layer: 'A' = ACT, 'V' = DVE
EV = {"xt": "V", "h1": "A", "h2a": "V", "h2b": "A", "g1": "A", "g2": "V"}
# out-stage eviction engine per j-tile (whole [128,1024] op)
OEVW = list("AVAV" "AVAV" "AVAV" "AVAV")
# out DMA queue per j-tile: 'S' = SP, 'A' = ACT, 'P' = GPSIMD
OQ = list("SPSS" "SPSP" "SPSS" "SPSP")

MLP_BUFS = 2
SM_BUFS = 2
UP_BUFS = 2


def round_f32r(x: np.ndarray) -> np.ndarray:
    """fp32 -> fp32r rounding (keep 11 explicit mantissa bits, RNE)."""
    u = np.ascontiguousarray(x, dtype=np.float32).view(np.uint32)
    keep = np.uint32(0xFFFFF000)
    half = np.uint32(0x800)
    lsb = (u >> np.uint32(12)) & np.uint32(1)
    r = (u + half - np.uint32(1) + lsb) & keep
    return r.view(np.float32)


def basis_coarse() -> np.ndarray:
    """Unnormalized Gaussian basis at S uniform t-samples: [P, S].
    Row-normalization of the reference basis cancels in num/den."""
    t = np.linspace(0.0, 1.0, S, dtype=np.float64)
    centers = np.arange(P, dtype=np.float64) / (P - 1)
    sigma = 1.0 / P
    z = t[None, :] - centers[:, None]
    return np.exp(-(z * z) / (2.0 * sigma * sigma)).astype(np.float32)


def interp_matrix() -> np.ndarray:
    """Cubic Lagrange interpolation weights from S uniform t-samples to the
    F output t-points: [S, F]."""
    t_s = np.linspace(0.0, 1.0, S)
    t_f = np.linspace(0.0, 1.0, F)
    I = np.zeros((S, F), np.float64)
    for fi, t in enumerate(t_f):
        k = np.searchsorted(t_s, t) - 1
        k = int(np.clip(k, 1, S - 3))
        idx = [k - 1, k, k + 1, k + 2]
        for a in range(4):
            w = 1.0
            for bb in range(4):
                if a != bb:
                    w *= (t - t_s[idx[bb]]) / (t_s[idx[a]] - t_s[idx[bb]])
            I[idx[a], fi] += w
    return I.astype(np.float32)


def build_program():
    nc = bacc.Bacc()
    x_in = nc.declare_dram_parameter("x", [BC, D], F32R, isOutput=False)
    wr_in = nc.declare_dram_parameter("wr", [128, C_R], F32R, isOutput=False)
    wf_in = nc.declare_dram_parameter("wf", [128, C_F], F32, isOutput=False)
    im_in = nc.declare_dram_parameter("im", [S, F], F32R, isOutput=False)
    out = nc.declare_dram_parameter("out", [BC, F], F32, isOutput=True)

    with tile.TileContext(nc) as tc, ExitStack() as ctx:
        cpool = ctx.enter_context(tc.tile_pool(name="const", bufs=1))
        wpool = ctx.enter_context(tc.tile_pool(name="work", bufs=2))
        opool = ctx.enter_context(tc.tile_pool(name="outp", bufs=4))
        ppool = ctx.enter_context(tc.tile_pool(name="ps", bufs=1, space="PSUM"))

        wr = cpool.tile([128, C_R], F32R)
        wf = cpool.tile([128, C_F], F32)
        im = cpool.tile([S, F], F32R)
        xall = cpool.tile([128, BC], F32R)

        def x_dma(eng, r0, r1):
            eng.dma_start(
                xall[:, r0:r1].rearrange("p (c d) -> p c d", c=(r1 - r0) // 128),
                x_in[r0:r1, :].rearrange("(c p) d -> p c d", p=128),
            )

        # fill-phase DMAs. ACT: ident -> biases -> weights -> interp matrix
        # (each needed successively later); SP and Pool split x.
        nc.scalar.dma_start(wr[:, _C_IDR:_C_IDR + 128], wr_in[:, _C_IDR:_C_IDR + 128])
        x_dma(nc.sync, 0, 512)
        x_dma(nc.gpsimd, 1024, 1536)
        nc.scalar.dma_start(wf[:], wf_in[:])
        x_dma(nc.sync, 512, 1024)
        x_dma(nc.gpsimd, 1536, 2048)
        nc.scalar.dma_start(wr[:, 0:_C_IDR], wr_in[:, 0:_C_IDR])
        nc.scalar.dma_start(im[:], im_in[:])

        def mm(out_ap, lhsT, rhs, start=True, stop=True):
            nc.tensor.matmul(out_ap, lhsT, rhs, start=start, stop=stop)

        def evict(eng, out_ap, in_ap, bias=None, act=None):
            """PSUM -> SBUF eviction with optional bias+activation."""
            if eng == "A":
                if act is None:
                    nc.scalar.activation(out_ap, in_ap, AF.Copy)
                else:
                    nc.scalar.activation(out_ap, in_ap, act, bias=bias)
            else:
                e = nc.vector if eng == "V" else nc.gpsimd
                if act is None and bias is None:
                    e.tensor_scalar(out=out_ap, in0=in_ap, scalar1=0.0,
                                    scalar2=None, op0=ALU.add)
                elif act == AF.Relu:
                    e.tensor_scalar(out=out_ap, in0=in_ap, scalar1=bias,
                                    scalar2=0.0, op0=ALU.add, op1=ALU.max)
                else:
                    raise ValueError((eng, act))

        ratios = [None] * NBLK

        def emit_mlp(b):
            x0 = NB * b
            xtp = ppool.tile([128, NB], F32R, tag="mlp", bufs=MLP_BUFS,
                             name=f"xtp{b}")
            for c in range(NB // 128):
                nc.tensor.matmul(
                    xtp[:, 128 * c:128 * (c + 1)],
                    xall[:, x0 + 128 * c:x0 + 128 * (c + 1)],
                    wr[:, _C_IDR:_C_IDR + 128],
                    is_transpose=True,
                    start=(c == 0), stop=(c == NB // 128 - 1),
                )
            xt = wpool.tile([128, NB], F32R, tag="xt", name=f"xt{b}")
            evict(EV["xt"], xt[:], xtp[:].bitcast(F32))

            h1p = ppool.tile([128, NB], F32, tag="mlp", bufs=MLP_BUFS,
                             name=f"h1p{b}")
            mm(h1p[:], wr[:, _C_W1T:_C_W1T + 128], xt[:])
            h1 = wpool.tile([128, NB], F32R, tag="h1", name=f"h1{b}")
            evict(EV["h1"], h1[:], h1p[:], bias=wf[:, _C_B1:_C_B1 + 1],
                  act=AF.Relu)

            h2ap = ppool.tile([128, NB], F32, tag="mlp", bufs=MLP_BUFS,
                              name=f"h2ap{b}")
            mm(h2ap[:], wr[:, _C_W2T:_C_W2T + 128], h1[:])
            h2a = wpool.tile([128, NB], F32R, tag="h2a", name=f"h2a{b}")
            evict(EV["h2a"], h2a[:], h2ap[:], bias=wf[:, _C_B2A:_C_B2A + 1],
                  act=AF.Relu)

            h2bp = ppool.tile([128, NB], F32, tag="mlp", bufs=MLP_BUFS,
                              name=f"h2bp{b}")
            mm(h2bp[:], wr[:, _C_W2T + 128:_C_W2T + 256], h1[:])
            h2b = wpool.tile([128, NB], F32R, tag="h2b", name=f"h2b{b}")
            evict(EV["h2b"], h2b[:], h2bp[:], bias=wf[:, _C_B2B:_C_B2B + 1],
                  act=AF.Relu)

            g1p = ppool.tile([64, NB], F32, tag="mlp", bufs=MLP_BUFS,
                             name=f"g1p{b}")
            mm(g1p[:], wr[:, _C_WW1T:_C_WW1T + 64], xt[:])
            g1 = wpool.tile([64, NB], F32R, tag="g1", name=f"g1{b}")
            evict(EV["g1"], g1[:], g1p[:], bias=wf[0:64, _C_WB1:_C_WB1 + 1],
                  act=AF.Relu)

            g2p = ppool.tile([128, NB], F32, tag="mlp", bufs=MLP_BUFS,
                             name=f"g2p{b}")
            mm(g2p[:], wr[0:64, _C_WW2T:_C_WW2T + 128], g1[:])
            g2 = wpool.tile([128, NB], F32R, tag="g2", name=f"g2{b}")
            evict(EV["g2"], g2[:], g2p[:], bias=wf[:, _C_WB2:_C_WB2 + 1],
                  act=AF.Relu)

            cpp = ppool.tile([20, NB], F32, tag="sm", bufs=SM_BUFS,
                             name=f"cpp{b}")
            mm(cpp[:], wr[:, _C_W3T:_C_W3T + 20], h2a[:], stop=False)
            mm(cpp[:], wr[:, _C_W3T + 20:_C_W3T + 40], h2b[:], start=False)
            cp = wpool.tile([20, NB], F32R, tag="cp", name=f"cp{b}")
            nc.scalar.activation(cp[:], cpp[:], AF.Tanh,
                                 bias=wf[0:20, _C_B3:_C_B3 + 1])

            wlp = ppool.tile([10, NB], F32, tag="sm", bufs=SM_BUFS,
                             name=f"wlp{b}")
            mm(wlp[:], wr[:, _C_WW3T:_C_WW3T + 10], g2[:])
            we2 = wpool.tile([10, NB], F32R, tag="we2", name=f"we2{b}")
            nc.scalar.activation(we2[:], wlp[:], AF.Exp,
                                 bias=wf[0:10, _C_WB3:_C_WB3 + 1])

            pairp = ppool.tile([10, NB], F32, tag="sm", bufs=SM_BUFS,
                               name=f"pairp{b}")
            mm(pairp[:], wr[0:20, _C_P20:_C_P20 + 10], cp[:])
            we1 = wpool.tile([10, NB], F32R, tag="we1", name=f"we1{b}")
            nc.vector.tensor_tensor(out=we1[:], in0=pairp[:],
                                    in1=we2[:].bitcast(F32), op=ALU.mult)

            ndn = ppool.tile([64, NB], F32, tag="sm", bufs=SM_BUFS,
                             name=f"ndn{b}")
            mm(ndn[:], wr[0:10, _C_BTC:_C_BTC + S], we1[:])
            ndd = ppool.tile([64, NB], F32, tag="sm", bufs=SM_BUFS,
                             name=f"ndd{b}")
            mm(ndd[:], wr[0:10, _C_BTC:_C_BTC + S], we2[:])
            rec = wpool.tile([S, NB], F32, tag="rec", name=f"rec{b}")
            nc.vector.reciprocal_approx_fast(out=rec[:], in_=ndd[:])
            ratio = wpool.tile([S, NB], F32R, tag="ratio", name=f"ratio{b}")
            nc.vector.tensor_tensor(out=ratio[:], in0=ndn[:], in1=rec[:],
                                    op=ALU.mult)
            ratios[b] = ratio

        def emit_out(b):
            x0 = NB * b
            ratio = ratios[b]
            for jj in range(NB // 128):
                j = (NB // 128) * b + jj
                lhsT = ratio[:, 128 * jj:128 * (jj + 1)]
                up = ppool.tile([128, F], F32, tag="up", bufs=UP_BUFS,
                                name=f"up{j}")
                mm(up[:, 0:512], lhsT, im[:, 0:512])
                mm(up[:, 512:1024], lhsT, im[:, 512:1024])
                o = opool.tile([128, F], F32, tag="o", name=f"o{j}")
                evict(OEVW[j], o[:], up[:])
                q = {"S": nc.sync, "A": nc.scalar, "P": nc.gpsimd}[OQ[j]]
                q.dma_start(out[x0 + 128 * jj:x0 + 128 * (jj + 1), :], o[:])

        emit_mlp(0)
        for b in range(1, NBLK):
            emit_mlp(b)
            emit_out(b - 1)
        emit_out(NBLK - 1)

    nc.compile()
    return nc


def host_consts(cp_w1, cp_b1, cp_w2, cp_b2, cp_w3, cp_b3,
                w_w1, w_b1, w_w2, w_b2, w_w3, w_b3):
    wr = np.zeros((128, C_R), np.float32)
    wr[:, _C_W1T:_C_W1T + 128] = cp_w1.T       # [128,128]
    wr[:, _C_W2T:_C_W2T + 256] = cp_w2.T       # [128,256]
    w3t = cp_w3.T                              # [256,20]
    wr[:, _C_W3T:_C_W3T + 20] = w3t[0:128]
    wr[:, _C_W3T + 20:_C_W3T + 40] = w3t[128:256]
    wr[:, _C_WW1T:_C_WW1T + 64] = w_w1.T       # [128,64]
    wr[0:64, _C_WW2T:_C_WW2T + 128] = w_w2.T   # [64,128]
    wr[:, _C_WW3T:_C_WW3T + 10] = w_w3.T       # [128,10]
    p20 = np.zeros((20, 10), np.float32)
    for p in range(P):
        p20[2 * p, p] = 0.5
        p20[2 * p + 1, p] = 0.5
    wr[0:20, _C_P20:_C_P20 + 10] = p20
    wr[0:10, _C_BTC:_C_BTC + S] = basis_coarse()
    wr[:, _C_IDR:_C_IDR + 128] = np.eye(128, dtype=np.float32)
    wr = round_f32r(wr)

    im = round_f32r(interp_matrix())           # [S, F]

    wf = np.zeros((128, C_F), np.float32)
    wf[:, _C_B1] = cp_b1
    wf[:, _C_B2A] = cp_b2[0:128]
    wf[:, _C_B2B] = cp_b2[128:256]
    wf[0:20, _C_B3] = cp_b3
    wf[0:64, _C_WB1] = w_b1
    wf[:, _C_WB2] = w_b2
    wf[0:10, _C_WB3] = w_b3
    return wr, wf, im


_NC_CACHE = None


def get_program():
    global _NC_CACHE
    if _NC_CACHE is None:
        _NC_CACHE = build_program()
    return _NC_CACHE


def kernel(x, cp_w1, cp_b1, cp_w2, cp_b2, cp_w3, cp_b3,
           w_w1, w_b1, w_w2, w_b2, w_w3, w_b3, _return_raw=False):
    x = np.asarray(x, np.float32)
    wr, wf, im = host_consts(
        np.asarray(cp_w1, np.float32), np.asarray(cp_b1, np.float32),
        np.asarray(cp_w2, np.float32), np.asarray(cp_b2, np.float32),
        np.asarray(cp_w3, np.float32), np.asarray(cp_b3, np.float32),
        np.asarray(w_w1, np.float32), np.asarray(w_b1, np.float32),
        np.asarray(w_w2, np.float32), np.asarray(w_b2, np.float32),
        np.asarray(w_w3, np.float32), np.asarray(w_b3, np.float32))

    nc = get_program()
    in_maps = [
        {"x": np.ascontiguousarray(x[i * BC:(i + 1) * BC]),
         "wr": wr, "wf": wf, "im": im}
        for i in range(NCORES)
    ]
    res = run_bass_kernel_spmd(nc, in_maps, list(range(NCORES)))
    outs = [res.results[i]["out"] for i in range(NCORES)]
    full = np.concatenate(outs, axis=0)
    if _return_raw:
        return full, res
    return full


# revision 11
# speedup vs baseline: 1.2152x; 1.0756x over previous
"""Trainium2 Bass kernel for nn_CPWGenerator (B=16384, D=128, P=10, F=1024).

Data-parallel over batch across 8 NeuronCores (2048 rows/core). Per core:
  - feature-major 3-layer MLPs (control-point head + weight head)
  - softmax denominator cancels: out = (basis @ (e*cpm)) / (basis @ e)
  - KEY RESTRUCTURE vs the first version: the ratio num/den is evaluated at
    only S=64 coarse t-samples (basis row-normalization cancels in the
    ratio), then upsampled to F=1024 by a single PE matmul against a
    precomputed cubic-Lagrange interpolation matrix im [S, F]. The ratio is
    a sum of 10 Gaussians with sigma ~ 102 grid points, so cubic
    interpolation from 64 uniform samples adds < 2e-4 relative error
    (measured end-to-end: 5.2e-4 total vs reference, gate is 2e-2).
    This cuts the full-resolution elementwise work (reciprocal + multiply
    at [128, 1024] per tile) down to one divide at [64, 512] per block.
  - PSUM->SBUF evictions are spread across ACT, DVE and GPSIMD; output DMA
    is spread across the SP, ACT and GPSIMD queues.
Matmuls run as float32r (fp32 storage, 11-bit-mantissa operand rounding,
exact fp32 accumulation) at full PE rate.
"""
import sys
if "/opt/trn_rl_repo" not in sys.path:
    sys.path.insert(0, "/opt/trn_rl_repo")

from contextlib import ExitStack

import numpy as np

import concourse.bacc as bacc
import concourse.mybir as mybir
import concourse.tile as tile
from concourse.bass_utils import run_bass_kernel_spmd

F32 = mybir.dt.float32
F32R = mybir.dt.float32r
AF = mybir.ActivationFunctionType
ALU = mybir.AluOpType

# problem shapes (hardcoded per contest contract)
B, D, P, F = 16384, 128, 10, 1024
NCORES = 8
BC = B // NCORES          # rows per core = 2048
NB = 512                  # batch block
NBLK = BC // NB           # 4
S = 64                    # coarse t-samples for the ratio
EPS = 1e-8

# f32r const blob (wr) column offsets
_C_W1T = 0            # [128 x 128]
_C_W2T = 128          # [128 x 256]
_C_W3T = 384          # [128 x 40]  (W3Ta | W3Tb, 20 cols each)
_C_WW1T = 424         # [128 x 64]
_C_WW2T = 488         # [64  x 128]
_C_WW3T = 616         # [128 x 10]
_C_P20 = 626          # [20  x 10]  pairing matrix (0.5 per pair)
_C_BTC = 636          # [10  x 64]  unnormalized basis at the S t-samples
_C_IDR = 700          # [128 x 128] identity (for PE transpose)
C_R = 828

# fp32 bias blob (wf) columns
_C_B1 = 0
_C_B2A = 1
_C_B2B = 2
_C_B3 = 3             # rows 0-19
_C_WB1 = 4            # rows 0-63
_C_WB2 = 5
_C_WB3 = 6            # rows 0-9
C_F = 7

# ---- engine assignment tables (tunable) ----
# (GPSIMD cannot access PSUM — BIR verifier — so evictions are ACT/DVE only.)
# MLP eviction engine per layer: 'A' = ACT, 'V' = DVE
EV = {"xt": "V", "h1": "A", "h2a": "V", "h2b": "A", "g1": "A", "g2": "V"}
# out-stage eviction engine per j-tile (whole [128,1024] op)
OEVW = list("AVAV" "AVAV" "AVAV" "AVAV")
# out DMA queue per j-tile: 'S' = SP, 'A' = ACT, 'P' = GPSIMD
OQ = list("SPSS" "SPSP" "SPSS" "SPSP")

MLP_BUFS = 2
SM_BUFS = 2
UP_BUFS = 2


def round_f32r(x: np.ndarray) -> np.ndarray:
    """fp32 -> fp32r rounding (keep 11 explicit mantissa bits, RNE)."""
    u = np.ascontiguousarray(x, dtype=np.float32).view(np.uint32)
    keep = np.uint32(0xFFFFF000)
    half = np.uint32(0x800)
    lsb = (u >> np.uint32(12)) & np.uint32(1)
    r = (u + half - np.uint32(1) + lsb) & keep
    return r.view(np.float32)


def basis_coarse() -> np.ndarray:
    """Unnormalized Gaussian basis at S uniform t-samples: [P, S].
    Row-normalization of the reference basis cancels in num/den."""
    t = np.linspace(0.0, 1.0, S, dtype=np.float64)
    centers = np.arange(P, dtype=np.float64) / (P - 1)
    sigma = 1.0 / P
    z = t[None, :] - centers[:, None]
    return np.exp(-(z * z) / (2.0 * sigma * sigma)).astype(np.float32)


def interp_matrix() -> np.ndarray:
    """Cubic Lagrange interpolation weights from S uniform t-samples to the
    F output t-points: [S, F]."""
    t_s = np.linspace(0.0, 1.0, S)
    t_f = np.linspace(0.0, 1.0, F)
    I = np.zeros((S, F), np.float64)
    for fi, t in enumerate(t_f):
        k = np.searchsorted(t_s, t) - 1
        k = int(np.clip(k, 1, S - 3))
        idx = [k - 1, k, k + 1, k + 2]
        for a in range(4):
            w = 1.0
            for bb in range(4):
                if a != bb:
                    w *= (t - t_s[idx[bb]]) / (t_s[idx[a]] - t_s[idx[bb]])
            I[idx[a], fi] += w
    return I.astype(np.float32)


def build_program():
    nc = bacc.Bacc()
    x_in = nc.declare_dram_parameter("x", [BC, D], F32R, isOutput=False)
    wr_in = nc.declare_dram_parameter("wr", [128, C_R], F32R, isOutput=False)
    wf_in = nc.declare_dram_parameter("wf", [128, C_F], F32, isOutput=False)
    im_in = nc.declare_dram_parameter("im", [S, F], F32R, isOutput=False)
    out = nc.declare_dram_parameter("out", [BC, F], F32, isOutput=True)

    with tile.TileContext(nc) as tc, ExitStack() as ctx:
        cpool = ctx.enter_context(tc.tile_pool(name="const", bufs=1))
        wpool = ctx.enter_context(tc.tile_pool(name="work", bufs=2))
        opool = ctx.enter_context(tc.tile_pool(name="outp", bufs=4))
        ppool = ctx.enter_context(tc.tile_pool(name="ps", bufs=1, space="PSUM"))

        wr = cpool.tile([128, C_R], F32R)
        wf = cpool.tile([128, C_F], F32)
        im = cpool.tile([S, F], F32R)
        xall = cpool.tile([128, BC], F32R)

        def x_dma(eng, r0, r1):
            eng.dma_start(
                xall[:, r0:r1].rearrange("p (c d) -> p c d", c=(r1 - r0) // 128),
                x_in[r0:r1, :].rearrange("(c p) d -> p c d", p=128),
            )

        # fill-phase DMAs, ordered by first use; ACT stays DMA-free (it is an
        # eviction bottleneck engine). SP: ident -> x blocks -> interp matrix.
        # Pool: biases -> weights -> x blocks.
        nc.sync.dma_start(wr[:, _C_IDR:_C_IDR + 128],
                          wr_in[:, _C_IDR:_C_IDR + 128])
        nc.gpsimd.dma_start(wf[:], wf_in[:])
        x_dma(nc.sync, 0, 512)
        nc.gpsimd.dma_start(wr[:, 0:_C_IDR], wr_in[:, 0:_C_IDR])
        x_dma(nc.sync, 512, 1024)
        x_dma(nc.gpsimd, 1024, 1536)
        nc.sync.dma_start(im[:], im_in[:])
        x_dma(nc.gpsimd, 1536, 2048)

        def mm(out_ap, lhsT, rhs, start=True, stop=True):
            nc.tensor.matmul(out_ap, lhsT, rhs, start=start, stop=stop)

        def evict(eng, out_ap, in_ap, bias=None, act=None):
            """PSUM -> SBUF eviction with optional bias+activation."""
            if eng == "A":
                if act is None:
                    nc.scalar.activation(out_ap, in_ap, AF.Copy)
                else:
                    nc.scalar.activation(out_ap, in_ap, act, bias=bias)
            else:
                e = nc.vector if eng == "V" else nc.gpsimd
                if act is None and bias is None:
                    e.tensor_scalar(out=out_ap, in0=in_ap, scalar1=0.0,
                                    scalar2=None, op0=ALU.add)
                elif act == AF.Relu:
                    e.tensor_scalar(out=out_ap, in0=in_ap, scalar1=bias,
                                    scalar2=0.0, op0=ALU.add, op1=ALU.max)
                else:
                    raise ValueError((eng, act))

        ratios = [None] * NBLK
        pending = []          # j-tiles of the previous block awaiting emission

        def emit_j(b, jj):
            x0 = NB * b
            ratio = ratios[b]
            j = (NB // 128) * b + jj
            lhsT = ratio[:, 128 * jj:128 * (jj + 1)]
            up = ppool.tile([128, F], F32, tag="up", bufs=UP_BUFS,
                            name=f"up{j}")
            mm(up[:, 0:512], lhsT, im[:, 0:512])
            mm(up[:, 512:1024], lhsT, im[:, 512:1024])
            o = opool.tile([128, F], F32, tag="o", name=f"o{j}")
            evict(OEVW[j], o[:], up[:])
            q = {"S": nc.sync, "A": nc.scalar, "P": nc.gpsimd}[OQ[j]]
            q.dma_start(out[x0 + 128 * jj:x0 + 128 * (jj + 1), :], o[:])

        def tick():
            if pending:
                emit_j(*pending.pop(0))

        def emit_mlp(b):
            x0 = NB * b
            xtp = ppool.tile([128, NB], F32R, tag="mlp", bufs=MLP_BUFS,
                             name=f"xtp{b}")
            for c in range(NB // 128):
                nc.tensor.matmul(
                    xtp[:, 128 * c:128 * (c + 1)],
                    xall[:, x0 + 128 * c:x0 + 128 * (c + 1)],
                    wr[:, _C_IDR:_C_IDR + 128],
                    is_transpose=True,
                    start=(c == 0), stop=(c == NB // 128 - 1),
                )
            xt = wpool.tile([128, NB], F32R, tag="xt", name=f"xt{b}")
            evict(EV["xt"], xt[:], xtp[:].bitcast(F32))

            h1p = ppool.tile([128, NB], F32, tag="mlp", bufs=MLP_BUFS,
                             name=f"h1p{b}")
            mm(h1p[:], wr[:, _C_W1T:_C_W1T + 128], xt[:])
            h1 = wpool.tile([128, NB], F32R, tag="h1", name=f"h1{b}")
            evict(EV["h1"], h1[:], h1p[:], bias=wf[:, _C_B1:_C_B1 + 1],
                  act=AF.Relu)
            tick()

            h2ap = ppool.tile([128, NB], F32, tag="mlp", bufs=MLP_BUFS,
                              name=f"h2ap{b}")
            mm(h2ap[:], wr[:, _C_W2T:_C_W2T + 128], h1[:])
            h2a = wpool.tile([128, NB], F32R, tag="h2a", name=f"h2a{b}")
            evict(EV["h2a"], h2a[:], h2ap[:], bias=wf[:, _C_B2A:_C_B2A + 1],
                  act=AF.Relu)

            h2bp = ppool.tile([128, NB], F32, tag="mlp", bufs=MLP_BUFS,
                              name=f"h2bp{b}")
            mm(h2bp[:], wr[:, _C_W2T + 128:_C_W2T + 256], h1[:])
            h2b = wpool.tile([128, NB], F32R, tag="h2b", name=f"h2b{b}")
            evict(EV["h2b"], h2b[:], h2bp[:], bias=wf[:, _C_B2B:_C_B2B + 1],
                  act=AF.Relu)
            tick()

            g1p = ppool.tile([64, NB], F32, tag="mlp", bufs=MLP_BUFS,
                             name=f"g1p{b}")
            mm(g1p[:], wr[:, _C_WW1T:_C_WW1T + 64], xt[:])
            g1 = wpool.tile([64, NB], F32R, tag="g1", name=f"g1{b}")
            evict(EV["g1"], g1[:], g1p[:], bias=wf[0:64, _C_WB1:_C_WB1 + 1],
                  act=AF.Relu)

            g2p = ppool.tile([128, NB], F32, tag="mlp", bufs=MLP_BUFS,
                             name=f"g2p{b}")
            mm(g2p[:], wr[0:64, _C_WW2T:_C_WW2T + 128], g1[:])
            g2 = wpool.tile([128, NB], F32R, tag="g2", name=f"g2{b}")
            evict(EV["g2"], g2[:], g2p[:], bias=wf[:, _C_WB2:_C_WB2 + 1],
                  act=AF.Relu)
            tick()

            cpp = ppool.tile([20, NB], F32, tag="sm", bufs=SM_BUFS,
                             name=f"cpp{b}")
            mm(cpp[:], wr[:, _C_W3T:_C_W3T + 20], h2a[:], stop=False)
            mm(cpp[:], wr[:, _C_W3T + 20:_C_W3T + 40], h2b[:], start=False)
            cp = wpool.tile([20, NB], F32R, tag="cp", name=f"cp{b}")
            nc.scalar.activation(cp[:], cpp[:], AF.Tanh,
                                 bias=wf[0:20, _C_B3:_C_B3 + 1])

            wlp = ppool.tile([10, NB], F32, tag="sm", bufs=SM_BUFS,
                             name=f"wlp{b}")
            mm(wlp[:], wr[:, _C_WW3T:_C_WW3T + 10], g2[:])
            we2 = wpool.tile([10, NB], F32R, tag="we2", name=f"we2{b}")
            nc.scalar.activation(we2[:], wlp[:], AF.Exp,
                                 bias=wf[0:10, _C_WB3:_C_WB3 + 1])

            pairp = ppool.tile([10, NB], F32, tag="sm", bufs=SM_BUFS,
                               name=f"pairp{b}")
            mm(pairp[:], wr[0:20, _C_P20:_C_P20 + 10], cp[:])
            we1 = wpool.tile([10, NB], F32R, tag="we1", name=f"we1{b}")
            nc.vector.tensor_tensor(out=we1[:], in0=pairp[:],
                                    in1=we2[:].bitcast(F32), op=ALU.mult)
            tick()

            ndn = ppool.tile([64, NB], F32, tag="sm", bufs=SM_BUFS,
                             name=f"ndn{b}")
            mm(ndn[:], wr[0:10, _C_BTC:_C_BTC + S], we1[:])
            ndd = ppool.tile([64, NB], F32, tag="sm", bufs=SM_BUFS,
                             name=f"ndd{b}")
            mm(ndd[:], wr[0:10, _C_BTC:_C_BTC + S], we2[:])
            rec = wpool.tile([S, NB], F32, tag="rec", name=f"rec{b}")
            nc.vector.reciprocal_approx_fast(out=rec[:], in_=ndd[:])
            ratio = wpool.tile([S, NB], F32R, tag="ratio", name=f"ratio{b}")
            nc.vector.tensor_tensor(out=ratio[:], in0=ndn[:], in1=rec[:],
                                    op=ALU.mult)
            ratios[b] = ratio

        emit_mlp(0)
        for b in range(1, NBLK):
            pending.extend((b - 1, jj) for jj in range(NB // 128))
            emit_mlp(b)
            while pending:
                emit_j(*pending.pop(0))
        for jj in range(NB // 128):
            emit_j(NBLK - 1, jj)

    nc.compile()
    return nc


def host_consts(cp_w1, cp_b1, cp_w2, cp_b2, cp_w3, cp_b3,
                w_w1, w_b1, w_w2, w_b2, w_w3, w_b3):
    wr = np.zeros((128, C_R), np.float32)
    wr[:, _C_W1T:_C_W1T + 128] = cp_w1.T       # [128,128]
    wr[:, _C_W2T:_C_W2T + 256] = cp_w2.T       # [128,256]
    w3t = cp_w3.T                              # [256,20]
    wr[:, _C_W3T:_C_W3T + 20] = w3t[0:128]
    wr[:, _C_W3T + 20:_C_W3T + 40] = w3t[128:256]
    wr[:, _C_WW1T:_C_WW1T + 64] = w_w1.T       # [128,64]
    wr[0:64, _C_WW2T:_C_WW2T + 128] = w_w2.T   # [64,128]
    wr[:, _C_WW3T:_C_WW3T + 10] = w_w3.T       # [128,10]
    p20 = np.zeros((20, 10), np.float32)
    for p in range(P):
        p20[2 * p, p] = 0.5
        p20[2 * p + 1, p] = 0.5
    wr[0:20, _C_P20:_C_P20 + 10] = p20
    wr[0:10, _C_BTC:_C_BTC + S] = basis_coarse()
    wr[:, _C_IDR:_C_IDR + 128] = np.eye(128, dtype=np.float32)
    wr = round_f32r(wr)

    im = round_f32r(interp_matrix())           # [S, F]

    wf = np.zeros((128, C_F), np.float32)
    wf[:, _C_B1] = cp_b1
    wf[:, _C_B2A] = cp_b2[0:128]
    wf[:, _C_B2B] = cp_b2[128:256]
    wf[0:20, _C_B3] = cp_b3
    wf[0:64, _C_WB1] = w_b1
    wf[:, _C_WB2] = w_b2
    wf[0:10, _C_WB3] = w_b3
    return wr, wf, im


_NC_CACHE = None


def get_program():
    global _NC_CACHE
    if _NC_CACHE is None:
        _NC_CACHE = build_program()
    return _NC_CACHE


def kernel(x, cp_w1, cp_b1, cp_w2, cp_b2, cp_w3, cp_b3,
           w_w1, w_b1, w_w2, w_b2, w_w3, w_b3, _return_raw=False):
    x = np.asarray(x, np.float32)
    wr, wf, im = host_consts(
        np.asarray(cp_w1, np.float32), np.asarray(cp_b1, np.float32),
        np.asarray(cp_w2, np.float32), np.asarray(cp_b2, np.float32),
        np.asarray(cp_w3, np.float32), np.asarray(cp_b3, np.float32),
        np.asarray(w_w1, np.float32), np.asarray(w_b1, np.float32),
        np.asarray(w_w2, np.float32), np.asarray(w_b2, np.float32),
        np.asarray(w_w3, np.float32), np.asarray(w_b3, np.float32))

    nc = get_program()
    in_maps = [
        {"x": np.ascontiguousarray(x[i * BC:(i + 1) * BC]),
         "wr": wr, "wf": wf, "im": im}
        for i in range(NCORES)
    ]
    res = run_bass_kernel_spmd(nc, in_maps, list(range(NCORES)))
    outs = [res.results[i]["out"] for i in range(NCORES)]
    full = np.concatenate(outs, axis=0)
    if _return_raw:
        return full, res
    return full
